# revision 39
# baseline (speedup 1.0000x reference)
"""BertLinearSelfAttention on 8 Trainium2 NeuronCores.

Problem (per reference):
  q = hs @ Wq.T + bq ; k = hs @ Wk.T + bk ; v = hs @ Wv.T + bv   (B,S,D)
  per head: scores = q @ k.T ; probs = scores * (mask >= 0) ; ctx = probs @ v
  B=2, S=2048, D=1024, H=16, HD=64. No softmax, binary key mask.

Key algebraic move: WITHOUT softmax the attention is linear in the
scores, so it reassociates:
  ctx_h = (q_h @ k_h.T * mask) @ v_h = q_h @ A_h,
  A_h = k_h.T @ diag(mask) @ v_h   -- a tiny [64, 64] matrix per head.
The S x S_k probs matrix is never materialized.

Sharding: core c = 4*b + g handles batch b and head group g (4 heads,
256 output features). SPMD program, host-side gather.

Host-side prep (free, like weight transposes): x cast to fp16,
transposed, and column-PERMUTED so the valid (mask>=0) keys come
first; all DRAM tensors are pre-packed into the EXACT SBUF layout so
every DMA is a contiguous [128, N] copy (8 fat descriptors per SDMA
engine instead of ~1k 256B ones -- delivery at ~358 GB/s instead of
~190).  Because keys are now a prefix of the query stream, KV chunks
4-7 are read straight out of the xq strip-1 image; only chunks 0-3
are duplicated (1 MB) so the K|V pipeline can start before a full
1 MB strip lands.  The device handles 8 key chunks (1024 keys); the
few valid keys beyond that are folded into A host-side as exact
rank-1 updates (same trick the baseline used for its 9-chunk CAP).

Device program per core:
  1) K|V: per key chunk sc (128 keys): kv[sc] [128, 512] =
     X_chunk @ [Wk.T | Wv.T], drained fp16 with *kvm slot mask.
     A-block matmuls (K_pair.T @ V_pair, [128,128] accumulating over
     chunks) lag one chunk behind the drains.
  2) qT: weight-stationary projection over 5 strips (512x3 + 256x2 --
     the mini final strips halve the serial drain+DMA+receipt tail).
  3) ctx: block-diagonal A [128,128] per head pair gives
     ctxT [128, w] = A.T @ qT; ctx lags one strip behind qT drains.
DMA order = consumption order on the sync queue (the early window
sustains only ~130-250 GB/s vs ~350 steady, so the first-matmul gate
is 640KB and granularity stays >=256KB -- DMA issue costs ~0.65us
apiece, so fine-graining backfires); small tensors on the scalar
queue.  An accumulating dummy-matmul run warms the PE clock (HAM
1.2->2.4GHz) while the first transfers land, sized so HAM flips
before the first real matmul.
Dtypes fp16 with fp32 PSUM accumulation; rel err ~7e-4 (tol 2e-2).
Measured: ~49.8-51.3us (baseline 56.6us); best runs 49.8-50.3.
Breakdown: ~4us counted preamble + ~4.5us first-data (HBM
early-window, high run-to-run variance) + ~31us PE (fp16 roofline
for this decomposition: 71680 moving cycles @2.4GHz) + ~3us out-tail
+ ~7us fixed epilogue (per-engine semaphore-bank zeroing sweep).
The kvhead kc-pair interleave pulls the first real matmul from
~13.2us to ~11.4us by letting the K|V pipeline's enabled-work curve
track the early delivery curve block-by-block.
"""
import numpy as np
import concourse.bass as bass
import concourse.mybir as mybir
import concourse.tile as tile
from concourse import bacc
from concourse.bass import ts
from concourse.bass_utils import run_bass_kernel_spmd

f32 = mybir.dt.float32
fp16 = mybir.dt.float16
AF = mybir.ActivationFunctionType

B = 2
S = 2048
D = 1024
DL = 256          # output features per core (4 heads x 64)
KC = D // 128     # 8 contraction chunks
MC = DL // 128    # 2 head pairs
SQW = 512         # sequence strip width
NSQ = S // SQW    # 4 strips
SKV = 8           # device key chunks; overflow keys folded host-side
CAP = SKV * 128
N_CORES = 8
N_WARM = 32       # dummy matmuls bridging the PE clock (HAM) window
                  # from engine-ready (~7.2us) to first-data (~10.6us);
                  # 32 cold dummies = ~3.4us of sustained busy, which
                  # flips HAM to 2.4GHz right as real matmuls start

_cache = {}
_last_perms = None


def _build(with_bias):
    nc = bacc.Bacc("TRN2", target_bir_lowering=False, debug=False,
                   num_devices=N_CORES)
    # All inputs pre-packed host-side into the exact SBUF layout:
    # every DMA below is a contiguous [128, N] copy.
    # xq col layout: s*4096 + kc*512 + q   (q = permuted query pos)
    XQ = nc.declare_dram_parameter("xq", [128, NSQ * KC * SQW], fp16,
                                   isOutput=False)
    # kvhead: one block per PIECE (kc range): wkv cols for those kcs
    # followed by chunk-0 cols for those kcs.  Delivered as one DMA
    # per piece in exact consumption order (160KB, 160KB, then 320KB
    # x3), so the K|V pipeline's enabled-work curve tracks the (slow)
    # early DMA delivery curve instead of waiting on one big prefix.
    KVHEAD = nc.declare_dram_parameter("kvhead", [128, KC * 640], fp16,
                                       isOutput=False)
    # chunks 1-3: col (sc-1)*1024 + kc*128 + j (chunk 0 lives in
    # kvhead; chunks 4-7 alias into the xq strip-1 image)
    XKV3 = nc.declare_dram_parameter("xkv3", [128, 3 * KC * 128], fp16,
                                     isOutput=False)
    WQT = nc.declare_dram_parameter("wqt", [128, KC * DL], fp16,
                                    isOutput=False)
    # col 0..SKV-1: kvm[p, sc] = 1.0 iff key slot sc*128+p is valid;
    # cols SKV..: host-folded A contribution of keys beyond the device
    # CAP (block-diagonal layout matching A_sb; zeros when no overflow)
    KVMAX = nc.declare_dram_parameter("kvmax", [128, SKV + MC * 128], f32,
                                      isOutput=False)
    if with_bias:
        BQ2 = nc.declare_dram_parameter("bq2", [128, MC], f32, isOutput=False)
        BKV = nc.declare_dram_parameter("bkv", [1, 2 * DL], fp16,
                                        isOutput=False)
        ONE = nc.declare_dram_parameter("ones", [1, 128], fp16, isOutput=False)
    # out[p, s*1024 + hp*512 + q] = ctxT[feature hp*128+p, seq s*512+q]
    OUT = nc.declare_dram_parameter("out", [128, 2 * S], fp16, isOutput=True)

    with tile.TileContext(nc) as tc:
        with tc.tile_pool(name="sb", bufs=1) as sb, \
             tc.tile_pool(name="stg", bufs=6) as stg, \
             tc.tile_pool(name="pkv", bufs=2, space="PSUM") as pkv, \
             tc.tile_pool(name="pA", bufs=2, space="PSUM") as pA, \
             tc.tile_pool(name="pQ", bufs=2, space="PSUM") as pQ:

            # kc pieces: two singles (tiny first-matmul gate), then pairs
            PIECES = [(0, 1), (1, 2), (2, 4), (4, 6), (6, 8)]
            PBASE = []
            acc = 0
            for s0, e0 in PIECES:
                PBASE.append(acc)
                acc += (e0 - s0) * 640

            kvhead = sb.tile([128, KC * 640], fp16, tag="kvhead")
            xkv = sb.tile([128, 3 * KC * 128], fp16, tag="xkv")
            xq = sb.tile([128, NSQ * KC * SQW], fp16, tag="xq")
            wqt = sb.tile([128, KC * DL], fp16, tag="wqt")

            # ---- DMA in, consumption order, contiguous [128,N] each.
            # The early DMA window sustains only ~130-250 GB/s (vs
            # ~350 steady), so the K|V phase start is delivery-bound:
            # the first matmul's gate is one 320KB block and each
            # kc-pair's data follows in its own block.  Granularity
            # stays >=256KB and the total issue count ~12 (issue cost
            # is ~0.65us apiece, so finer-graining backfires).
            for p, (s0, e0) in enumerate(PIECES):
                w0 = (e0 - s0) * 640
                nc.sync.dma_start(kvhead[:, PBASE[p]:PBASE[p] + w0],
                                  KVHEAD.ap()[:, PBASE[p]:PBASE[p] + w0])
            for c in (0, 1, 2):
                nc.sync.dma_start(xkv[:, c * 1024:(c + 1) * 1024],
                                  XKV3.ap()[:, c * 1024:(c + 1) * 1024])
            # strip 1 feeds KV chunks 4-7, so it precedes strip 0
            nc.sync.dma_start(xq[:, 4096:8192], XQ.ap()[:, 4096:8192])
            nc.sync.dma_start(wqt[:], WQT.ap()[:, :])
            nc.sync.dma_start(xq[:, 0:4096], XQ.ap()[:, 0:4096])
            nc.sync.dma_start(xq[:, 8192:16384], XQ.ap()[:, 8192:16384])
            kvmax = sb.tile([128, SKV + MC * 128], f32, tag="kvmax")
            nc.scalar.dma_start(kvmax[:], KVMAX[:, :])
            if with_bias:
                bq2 = sb.tile([128, MC], f32, tag="bq2")
                nc.scalar.dma_start(bq2[:], BQ2[:, :])
                bkv = sb.tile([1, 2 * DL], fp16, tag="bkv")
                nc.scalar.dma_start(bkv[:], BKV[:, :])
                ones = sb.tile([1, 128], fp16, tag="ones")
                nc.scalar.dma_start(ones[:], ONE[:, :])

            kv_sb = sb.tile([128, SKV * 512], fp16, tag="kv_sb")
            qT = [sb.tile([128, S], fp16, tag=f"qT{mc}", name=f"qT{mc}")
                  for mc in range(MC)]
            # A_sb holds one BLOCK-DIAGONAL [128,128] matrix per head
            # pair (off-diagonal cross-head blocks stay zero).
            A_sb = sb.tile([128, MC * 128], fp16, tag="A_sb")
            nc.vector.memset(A_sb[:], 0)

            # ---- PE warm-up: one long accumulating matmul group on a
            # zeroed tile keeps the tensor engine busy through the HAM
            # activity window while the first transfers land.
            warm = sb.tile([128, 128], fp16, tag="warm")
            nc.vector.memset(warm[:], 0)
            wp = pQ.tile([128, 128], f32, tag="ct", name="warm_ps")
            for i in range(N_WARM):
                nc.tensor.matmul(wp[:], warm[:], warm[:],
                                 start=(i == 0), stop=(i == N_WARM - 1))

            eng = 0

            def drain(dst_ap, src_ap, bias=None, scale=None):
                nonlocal eng
                if eng == 0:
                    if bias is not None:
                        nc.vector.tensor_scalar_add(dst_ap, src_ap, bias)
                    elif scale is not None:
                        nc.vector.tensor_scalar_mul(dst_ap, src_ap, scale)
                    else:
                        nc.vector.tensor_copy(dst_ap, src_ap)
                else:
                    if bias is not None:
                        nc.scalar.add(dst_ap, src_ap, bias)
                    elif scale is not None:
                        nc.scalar.activation(dst_ap, src_ap, AF.Copy,
                                             scale=scale)
                    else:
                        nc.scalar.copy(dst_ap, src_ap)
                eng ^= 1

            def _piece(kc):
                for p, (s0, e0) in enumerate(PIECES):
                    if s0 <= kc < e0:
                        return p, s0, e0
                raise AssertionError

            def wkv_sl(kc):
                p, s0, e0 = _piece(kc)
                off = PBASE[p] + (kc - s0) * 512
                return kvhead[:, off:off + 512]

            def xsl(sc, kc):
                # key chunk sc, contraction chunk kc: [128, 128] slice
                if sc == 0:
                    p, s0, e0 = _piece(kc)
                    off = PBASE[p] + (e0 - s0) * 512 + (kc - s0) * 128
                    return kvhead[:, off:off + 128]
                if sc < 4:
                    off = (sc - 1) * 1024 + kc * 128
                    return xkv[:, off:off + 128]
                off = 4096 + kc * 512 + (sc - 4) * 128
                return xq[:, off:off + 128]

            # ---- phase 1+2: K|V chunks with lagged A accumulation ----
            A_ps = [pA.tile([128, 128], f32, tag="A", name=f"A{hp}")
                    for hp in range(MC)]

            def emit_A(sc):
                for hp in range(MC):
                    nc.tensor.matmul(
                        A_ps[hp][:],
                        kv_sb[:, sc * 512 + hp * 128:
                              sc * 512 + hp * 128 + 128],
                        kv_sb[:, sc * 512 + 256 + hp * 128:
                              sc * 512 + 256 + hp * 128 + 128],
                        start=(sc == 0), stop=(sc == SKV - 1))

            pend_A = None
            for sc in range(SKV):
                kvp = pkv.tile([128, 2 * DL], f32, tag="kvp")
                if with_bias:
                    nc.tensor.matmul(kvp[:], ones[:], bkv[:],
                                     start=True, stop=False)
                for kc in range(KC):
                    nc.tensor.matmul(
                        kvp[:],
                        xsl(sc, kc),
                        wkv_sl(kc),
                        start=(kc == 0 and not with_bias),
                        stop=(kc == KC - 1))
                drain(kv_sb[:, ts(sc, 512)], kvp[:],
                      scale=kvmax[:, sc:sc + 1])
                if pend_A is not None:
                    emit_A(pend_A)
                pend_A = sc

            # ---- phase 3+4: qT strips with ctx lagged one strip ------
            # Strips: three 512-wide, then two 256-wide so the final
            # ctx piece (the serial tail: drain + out-DMA + receipt)
            # covers half the bytes.
            STRIPS = [(0, 512), (512, 512), (1024, 512),
                      (1536, 256), (1792, 256)]

            def xq_col(off, kc):
                s0, within = divmod(off, 512)
                return s0 * 4096 + kc * 512 + within

            def out_col(off, hp):
                s0, within = divmod(off, 512)
                return s0 * 1024 + hp * 512 + within

            def emit_ctx(si, split=False):
                off, w = STRIPS[si]
                # the final strip's ctx draws PSUM from the (long idle)
                # kv pool so it never waits on the ct-buffer rotation
                stage = stg.tile([128, 2 * SQW], fp16, tag="st")
                for hp in range(MC):
                    if split:
                        ct = pkv.tile([128, 2 * DL], f32, tag="kvp",
                                      name="ct_last")
                    else:
                        ct = pQ.tile([128, SQW], f32, tag="ct", name="ct")
                    nc.tensor.matmul(
                        ct[:, 0:w],
                        A_sb[:, ts(hp, 128)],
                        qT[hp][:, off:off + w],
                        start=True, stop=True)
                    drain(stage[:, hp * w:(hp + 1) * w], ct[:, 0:w])
                    if split:
                        # ship per head pair on separate queues so the
                        # final issues and receipts overlap
                        q = nc.sync if hp == 0 else nc.scalar
                        q.dma_start(
                            OUT[:, out_col(off, hp):out_col(off, hp) + w],
                            stage[:, hp * w:(hp + 1) * w])
                if not split:
                    for hp in range(MC):
                        # the penultimate strip's second piece goes out
                        # on gpsimd so the final strip's sync issue
                        # doesn't queue behind it
                        q = (nc.gpsimd if (hp == 1 and
                                           si == len(STRIPS) - 2)
                             else nc.sync)
                        q.dma_start(
                            OUT[:, out_col(off, hp):out_col(off, hp) + w],
                            stage[:, hp * w:(hp + 1) * w])

            for si, (off, w) in enumerate(STRIPS):
                for mc in range(MC):
                    qp = pQ.tile([128, SQW], f32, tag="qp")
                    for kc in range(KC):
                        nc.tensor.matmul(
                            qp[:, 0:w],
                            wqt[:, kc * DL + mc * 128:kc * DL + mc * 128 + 128],
                            xq[:, xq_col(off, kc):xq_col(off, kc) + w],
                            start=(kc == 0), stop=(kc == KC - 1))
                    if si == 0 and mc == 0:
                        emit_A(pend_A)
                        for hp in range(MC):
                            # only the per-head diagonal blocks (the
                            # cross-head blocks of A_ps are garbage and
                            # A_sb stays zero there), plus the host-
                            # folded overflow-key contribution
                            nc.vector.tensor_add(
                                A_sb[0:64, hp * 128:hp * 128 + 64],
                                A_ps[hp][0:64, 0:64],
                                kvmax[0:64, SKV + hp * 128:
                                      SKV + hp * 128 + 64])
                            nc.vector.tensor_add(
                                A_sb[64:128, hp * 128 + 64:(hp + 1) * 128],
                                A_ps[hp][64:128, 64:128],
                                kvmax[64:128, SKV + hp * 128 + 64:
                                      SKV + (hp + 1) * 128])
                    drain(qT[mc][:, off:off + w], qp[:, 0:w],
                          bias=(bq2[:, mc:mc + 1] if with_bias else None))
                if si > 0:
                    emit_ctx(si - 1)
            emit_ctx(len(STRIPS) - 1, split=True)

    nc.compile()
    return nc


def _get_nc(skv, with_bias):
    key = bool(with_bias)
    if key not in _cache:
        _cache[key] = _build(key)
    return _cache[key]


def _make_in_maps(hidden_states, attention_mask, Wq, bq, Wk, bk, Wv, bv):
    global _last_perms
    hs16 = np.asarray(hidden_states, dtype=np.float32).astype(np.float16)
    am = np.asarray(attention_mask, dtype=np.float32)
    bq = np.asarray(bq, np.float32)
    bk = np.asarray(bk, np.float32)
    bv = np.asarray(bv, np.float32)
    with_bias = bool(bq.any() or bk.any() or bv.any())

    valid = [np.nonzero(am[b, 0, 0, :] >= 0)[0] for b in range(B)]
    nmax = max(len(v) for v in valid)
    if nmax == 0:
        return None, with_bias, None   # all keys masked -> zero output

    xqs, xkvs, kvms, overs, perms = [], [], [], [], []
    for b in range(B):
        v = valid[b]
        inv = np.nonzero(am[b, 0, 0, :] < 0)[0]
        perm = np.concatenate([v, inv])          # valid keys first
        perms.append(perm)
        overs.append(v[CAP:])                    # folded host-side
        nv = min(len(v), CAP)
        kvm = (np.arange(CAP) < nv).astype(np.float32)
        kvms.append(np.ascontiguousarray(kvm.reshape(SKV, 128).T))
        xp = np.ascontiguousarray(hs16[b].T[:, perm])          # [D, S]
        xqs.append(np.ascontiguousarray(
            xp.reshape(KC, 128, NSQ, SQW).transpose(1, 2, 0, 3)
            .reshape(128, NSQ * KC * SQW)))
        xkvs.append(np.ascontiguousarray(
            xp[:, 0:512].reshape(KC, 128, 4, 128).transpose(1, 2, 0, 3)
            .reshape(128, 4 * KC * 128)))
    _last_perms = perms

    Wq = np.asarray(Wq, np.float32)
    Wk = np.asarray(Wk, np.float32)
    Wv = np.asarray(Wv, np.float32)

    def chunked(w):  # [1024, F] -> [128, KC*F] with col kc*F + m
        F = w.shape[1]
        return np.ascontiguousarray(
            w.reshape(KC, 128, F).transpose(1, 0, 2).reshape(128, KC * F)
            .astype(np.float16))

    in_maps = []
    for c in range(N_CORES):
        b, g = divmod(c, 4)
        sl = slice(g * DL, (g + 1) * DL)
        axm = np.zeros((128, MC * 128), np.float32)
        for j in overs[b]:
            x = hs16[b][j].astype(np.float32)
            kf = Wk[sl] @ x + bk[sl]
            vf = Wv[sl] @ x + bv[sl]
            for hp in range(MC):
                for h in range(2):
                    f = slice(hp * 128 + h * 64, hp * 128 + (h + 1) * 64)
                    axm[h * 64:(h + 1) * 64, f] += np.outer(kf[f], vf[f])
        wkv_img = chunked(np.concatenate([Wk[sl, :].T, Wv[sl, :].T],
                                         axis=1))
        c0_img = xkvs[b][:, 0:1024]
        pieces = []
        for s0, e0 in [(0, 1), (1, 2), (2, 4), (4, 6), (6, 8)]:
            pieces.append(wkv_img[:, s0 * 512:e0 * 512])
            pieces.append(c0_img[:, s0 * 128:e0 * 128])
        m = {
            "xq": xqs[b],
            "xkv3": np.ascontiguousarray(xkvs[b][:, 1024:4096]),
            "kvhead": np.ascontiguousarray(np.concatenate(pieces, axis=1)),
            "wqt": chunked(Wq[sl, :].T),
            "kvmax": np.ascontiguousarray(
                np.concatenate([kvms[b], axm], axis=1)),
        }
        if with_bias:
            m["bq2"] = np.ascontiguousarray(bq[sl].reshape(MC, 128).T)
            m["bkv"] = np.ascontiguousarray(
                np.concatenate([bk[sl], bv[sl]]).reshape(1, 2 * DL)
                .astype(np.float16))
            m["ones"] = np.ones((1, 128), np.float16)
        in_maps.append(m)
    return SKV, with_bias, in_maps


def _gather(results):
    out = np.empty((B, S, D), np.float32)
    for c in range(N_CORES):
        b, g = divmod(c, 4)
        # out dram [128, NSQ*2*512]: [p, (s, hp, q)] -> [s*512+q, hp*128+p]
        arr = results[c]["out"].reshape(128, NSQ, MC, SQW)
        tmp = arr.transpose(1, 3, 2, 0).reshape(S, DL).astype(np.float32)
        out[b, _last_perms[b], g * DL:(g + 1) * DL] = tmp
    return out


def run_sharded(skv, with_bias, in_maps, **kw):
    nc = _get_nc(skv, with_bias)
    return run_bass_kernel_spmd(nc, in_maps, core_ids=list(range(N_CORES)),
                                **kw)


def kernel(hidden_states, attention_mask, Wq, bq, Wk, bk, Wv, bv):
    skv, with_bias, in_maps = _make_in_maps(
        hidden_states, attention_mask, Wq, bq, Wk, bk, Wv, bv)
    if skv is None:
        return np.zeros((B, S, D), np.float32)
    res = run_sharded(skv, with_bias, in_maps)
    return _gather(res.results)


# revision 40
# speedup vs baseline: 1.0556x; 1.0556x over previous
"""BertLinearSelfAttention on 8 Trainium2 NeuronCores.

Problem (per reference):
  q = hs @ Wq.T + bq ; k = hs @ Wk.T + bk ; v = hs @ Wv.T + bv   (B,S,D)
  per head: scores = q @ k.T ; probs = scores * (mask >= 0) ; ctx = probs @ v
  B=2, S=2048, D=1024, H=16, HD=64. No softmax, binary key mask.

Key algebraic move: WITHOUT softmax the attention is linear in the
scores, so it reassociates:
  ctx_h = (q_h @ k_h.T * mask) @ v_h = q_h @ A_h,
  A_h = k_h.T @ diag(mask) @ v_h   -- a tiny [64, 64] matrix per head.
The S x S_k probs matrix is never materialized.

Sharding: core c = 4*b + g handles batch b and head group g (4 heads,
256 output features). SPMD program, host-side gather.

Host-side prep (free, like weight transposes): x cast to fp16,
transposed, and column-PERMUTED so the valid (mask>=0) keys come
first; all DRAM tensors are pre-packed into the EXACT SBUF layout so
every DMA is a contiguous [128, N] copy (8 fat descriptors per SDMA
engine instead of ~1k 256B ones -- delivery at ~358 GB/s instead of
~190).  Because keys are now a prefix of the query stream, KV chunks
4-7 are read straight out of the xq strip-1 image; only chunks 0-3
are duplicated (1 MB) so the K|V pipeline can start before a full
1 MB strip lands.  The device handles 8 key chunks (1024 keys); the
few valid keys beyond that are folded into A host-side as exact
rank-1 updates (same trick the baseline used for its 9-chunk CAP).

Device program per core:
  1) K|V: per key chunk sc (128 keys): kv[sc] [128, 512] =
     X_chunk @ [Wk.T | Wv.T], drained fp16 with *kvm slot mask.
     A-block matmuls (K_pair.T @ V_pair, [128,128] accumulating over
     chunks) lag one chunk behind the drains.
  2) qT: weight-stationary projection over 5 strips (512x3 + 256x2 --
     the mini final strips halve the serial drain+DMA+receipt tail).
  3) ctx: block-diagonal A [128,128] per head pair gives
     ctxT [128, w] = A.T @ qT; ctx lags one strip behind qT drains.
DMA order = consumption order on the sync queue (the early window
sustains only ~130-250 GB/s vs ~350 steady, so the first-matmul gate
is 640KB and granularity stays >=256KB -- DMA issue costs ~0.65us
apiece, so fine-graining backfires); small tensors on the scalar
queue.  An accumulating dummy-matmul run warms the PE clock (HAM
1.2->2.4GHz) while the first transfers land, sized so HAM flips
before the first real matmul.
Dtypes fp16 with fp32 PSUM accumulation; rel err ~7e-4 (tol 2e-2).
Measured: ~49.8-51.3us (baseline 56.6us); best runs 49.8-50.3.
Breakdown: ~4us counted preamble + ~4.5us first-data (HBM
early-window, high run-to-run variance) + ~31us PE (fp16 roofline
for this decomposition: 71680 moving cycles @2.4GHz) + ~3us out-tail
+ ~7us fixed epilogue (per-engine semaphore-bank zeroing sweep).
The kvhead kc-pair interleave pulls the first real matmul from
~13.2us to ~11.4us by letting the K|V pipeline's enabled-work curve
track the early delivery curve block-by-block.
"""
import numpy as np
import concourse.bass as bass
import concourse.mybir as mybir
import concourse.tile as tile
from concourse import bacc
from concourse.bass import ts
from concourse.bass_utils import run_bass_kernel_spmd

f32 = mybir.dt.float32
fp16 = mybir.dt.float16
AF = mybir.ActivationFunctionType

B = 2
S = 2048
D = 1024
DL = 256          # output features per core (4 heads x 64)
KC = D // 128     # 8 contraction chunks
MC = DL // 128    # 2 head pairs
SQW = 512         # sequence strip width
NSQ = S // SQW    # 4 strips
SKV = 8           # device key chunks; overflow keys folded host-side
CAP = SKV * 128
N_CORES = 8
N_WARM = 45       # dummy matmuls bridging the PE clock (HAM) window:
                  # 32 run cold (~3.4us, flipping HAM to 2.4GHz), the
                  # rest run warm to ~11.4us -- the earliest first-data
                  # ever observed -- so slow-delivery runs keep the PE
                  # busy-fraction high enough that HAM's MID window
                  # never re-throttles during the stall-paced early KV

_cache = {}
_last_perms = None


def _build(with_bias):
    nc = bacc.Bacc("TRN2", target_bir_lowering=False, debug=False,
                   num_devices=N_CORES)
    # All inputs pre-packed host-side into the exact SBUF layout:
    # every DMA below is a contiguous [128, N] copy.
    # xq col layout: s*4096 + kc*512 + q   (q = permuted query pos)
    XQ = nc.declare_dram_parameter("xq", [128, NSQ * KC * SQW], fp16,
                                   isOutput=False)
    # kvhead: 4 blocks, one per kc-pair j: cols j*1280+[0:1024) =
    # wkv kc(2j),kc(2j+1); [1024:1280) = chunk-0 kc(2j),kc(2j+1).
    # Delivered as four 320KB DMAs in exact consumption order, so the
    # K|V pipeline's enabled-work curve tracks the (slow) early DMA
    # delivery curve instead of waiting on one big prefix.
    KVHEAD = nc.declare_dram_parameter("kvhead", [128, 4 * 1280], fp16,
                                       isOutput=False)
    # chunks 1-3: col (sc-1)*1024 + kc*128 + j (chunk 0 lives in
    # kvhead; chunks 4-7 alias into the xq strip-1 image)
    XKV3 = nc.declare_dram_parameter("xkv3", [128, 3 * KC * 128], fp16,
                                     isOutput=False)
    WQT = nc.declare_dram_parameter("wqt", [128, KC * DL], fp16,
                                    isOutput=False)
    # col 0..SKV-1: kvm[p, sc] = 1.0 iff key slot sc*128+p is valid;
    # cols SKV..: host-folded A contribution of keys beyond the device
    # CAP (block-diagonal layout matching A_sb; zeros when no overflow)
    KVMAX = nc.declare_dram_parameter("kvmax", [128, SKV + MC * 128], f32,
                                      isOutput=False)
    if with_bias:
        BQ2 = nc.declare_dram_parameter("bq2", [128, MC], f32, isOutput=False)
        BKV = nc.declare_dram_parameter("bkv", [1, 2 * DL], fp16,
                                        isOutput=False)
        ONE = nc.declare_dram_parameter("ones", [1, 128], fp16, isOutput=False)
    # out[p, s*1024 + hp*512 + q] = ctxT[feature hp*128+p, seq s*512+q]
    OUT = nc.declare_dram_parameter("out", [128, 2 * S], fp16, isOutput=True)

    with tile.TileContext(nc) as tc:
        with tc.tile_pool(name="sb", bufs=1) as sb, \
             tc.tile_pool(name="stg", bufs=4) as stg, \
             tc.tile_pool(name="pkv", bufs=2, space="PSUM") as pkv, \
             tc.tile_pool(name="pA", bufs=2, space="PSUM") as pA, \
             tc.tile_pool(name="pQ", bufs=2, space="PSUM") as pQ:

            kvhead = sb.tile([128, 4 * 1280], fp16, tag="kvhead")
            xkv = sb.tile([128, 3 * KC * 128], fp16, tag="xkv")
            xq = sb.tile([128, NSQ * KC * SQW], fp16, tag="xq")
            wqt = sb.tile([128, KC * DL], fp16, tag="wqt")

            # ---- DMA in, consumption order, contiguous [128,N] each.
            # The early DMA window sustains only ~130-250 GB/s (vs
            # ~350 steady), so the K|V phase start is delivery-bound:
            # the first matmul's gate is one 320KB block and each
            # kc-pair's data follows in its own block.  Granularity
            # stays >=256KB and the total issue count ~12 (issue cost
            # is ~0.65us apiece, so finer-graining backfires).
            for j in range(4):
                nc.sync.dma_start(kvhead[:, j * 1280:(j + 1) * 1280],
                                  KVHEAD.ap()[:, j * 1280:(j + 1) * 1280])
            for c in (0, 1, 2):
                nc.sync.dma_start(xkv[:, c * 1024:(c + 1) * 1024],
                                  XKV3.ap()[:, c * 1024:(c + 1) * 1024])
            # strip 1 feeds KV chunks 4-7, so it precedes strip 0
            nc.sync.dma_start(xq[:, 4096:8192], XQ.ap()[:, 4096:8192])
            nc.sync.dma_start(wqt[:], WQT.ap()[:, :])
            nc.sync.dma_start(xq[:, 0:4096], XQ.ap()[:, 0:4096])
            nc.sync.dma_start(xq[:, 8192:16384], XQ.ap()[:, 8192:16384])
            kvmax = sb.tile([128, SKV + MC * 128], f32, tag="kvmax")
            nc.scalar.dma_start(kvmax[:], KVMAX[:, :])
            if with_bias:
                bq2 = sb.tile([128, MC], f32, tag="bq2")
                nc.scalar.dma_start(bq2[:], BQ2[:, :])
                bkv = sb.tile([1, 2 * DL], fp16, tag="bkv")
                nc.scalar.dma_start(bkv[:], BKV[:, :])
                ones = sb.tile([1, 128], fp16, tag="ones")
                nc.scalar.dma_start(ones[:], ONE[:, :])

            kv_sb = sb.tile([128, SKV * 512], fp16, tag="kv_sb")
            qT = [sb.tile([128, S], fp16, tag=f"qT{mc}", name=f"qT{mc}")
                  for mc in range(MC)]
            # A_sb holds one BLOCK-DIAGONAL [128,128] matrix per head
            # pair (off-diagonal cross-head blocks stay zero).
            A_sb = sb.tile([128, MC * 128], fp16, tag="A_sb")
            nc.vector.memset(A_sb[:], 0)

            # ---- PE warm-up: one long accumulating matmul group on a
            # zeroed tile keeps the tensor engine busy through the HAM
            # activity window while the first transfers land.
            warm = sb.tile([128, 128], fp16, tag="warm")
            nc.vector.memset(warm[:], 0)
            wp = pQ.tile([128, 128], f32, tag="ct", name="warm_ps")
            for i in range(N_WARM):
                nc.tensor.matmul(wp[:], warm[:], warm[:],
                                 start=(i == 0), stop=(i == N_WARM - 1))

            eng = 0

            def drain(dst_ap, src_ap, bias=None, scale=None):
                nonlocal eng
                if eng == 0:
                    if bias is not None:
                        nc.vector.tensor_scalar_add(dst_ap, src_ap, bias)
                    elif scale is not None:
                        nc.vector.tensor_scalar_mul(dst_ap, src_ap, scale)
                    else:
                        nc.vector.tensor_copy(dst_ap, src_ap)
                else:
                    if bias is not None:
                        nc.scalar.add(dst_ap, src_ap, bias)
                    elif scale is not None:
                        nc.scalar.activation(dst_ap, src_ap, AF.Copy,
                                             scale=scale)
                    else:
                        nc.scalar.copy(dst_ap, src_ap)
                eng ^= 1

            def wkv_sl(kc):
                j, r = divmod(kc, 2)
                return kvhead[:, j * 1280 + r * 512:
                              j * 1280 + (r + 1) * 512]

            def xsl(sc, kc):
                # key chunk sc, contraction chunk kc: [128, 128] slice
                if sc == 0:
                    j, r = divmod(kc, 2)
                    off = j * 1280 + 1024 + r * 128
                    return kvhead[:, off:off + 128]
                if sc < 4:
                    off = (sc - 1) * 1024 + kc * 128
                    return xkv[:, off:off + 128]
                off = 4096 + kc * 512 + (sc - 4) * 128
                return xq[:, off:off + 128]

            # ---- phase 1+2: K|V chunks with lagged A accumulation ----
            A_ps = [pA.tile([128, 128], f32, tag="A", name=f"A{hp}")
                    for hp in range(MC)]

            def emit_A(sc):
                for hp in range(MC):
                    nc.tensor.matmul(
                        A_ps[hp][:],
                        kv_sb[:, sc * 512 + hp * 128:
                              sc * 512 + hp * 128 + 128],
                        kv_sb[:, sc * 512 + 256 + hp * 128:
                              sc * 512 + 256 + hp * 128 + 128],
                        start=(sc == 0), stop=(sc == SKV - 1))

            pend_A = None
            for sc in range(SKV):
                kvp = pkv.tile([128, 2 * DL], f32, tag="kvp")
                if with_bias:
                    nc.tensor.matmul(kvp[:], ones[:], bkv[:],
                                     start=True, stop=False)
                for kc in range(KC):
                    nc.tensor.matmul(
                        kvp[:],
                        xsl(sc, kc),
                        wkv_sl(kc),
                        start=(kc == 0 and not with_bias),
                        stop=(kc == KC - 1))
                drain(kv_sb[:, ts(sc, 512)], kvp[:],
                      scale=kvmax[:, sc:sc + 1])
                if pend_A is not None:
                    emit_A(pend_A)
                pend_A = sc

            # ---- phase 3+4: qT strips with ctx lagged one strip ------
            # Strips: three 512-wide, then two 256-wide so the final
            # ctx piece (the serial tail: drain + out-DMA + receipt)
            # covers half the bytes.
            STRIPS = [(0, 512), (512, 512), (1024, 512),
                      (1536, 256), (1792, 256)]

            def xq_col(off, kc):
                s0, within = divmod(off, 512)
                return s0 * 4096 + kc * 512 + within

            def out_col(off, hp):
                s0, within = divmod(off, 512)
                return s0 * 1024 + hp * 512 + within

            def emit_ctx(si, split=False):
                off, w = STRIPS[si]
                # the final strip's ctx draws PSUM from the (long idle)
                # kv pool so it never waits on the ct-buffer rotation
                stage = stg.tile([128, 2 * SQW], fp16, tag="st")
                for hp in range(MC):
                    if split:
                        ct = pkv.tile([128, 2 * DL], f32, tag="kvp",
                                      name="ct_last")
                    else:
                        ct = pQ.tile([128, SQW], f32, tag="ct", name="ct")
                    nc.tensor.matmul(
                        ct[:, 0:w],
                        A_sb[:, ts(hp, 128)],
                        qT[hp][:, off:off + w],
                        start=True, stop=True)
                    drain(stage[:, hp * w:(hp + 1) * w], ct[:, 0:w])
                    if split:
                        # ship per head pair on separate queues so the
                        # final issues and receipts overlap
                        q = nc.sync if hp == 0 else nc.scalar
                        q.dma_start(
                            OUT[:, out_col(off, hp):out_col(off, hp) + w],
                            stage[:, hp * w:(hp + 1) * w])
                if not split:
                    for hp in range(MC):
                        # the penultimate strip's second piece goes out
                        # on gpsimd so the final strip's sync issue
                        # doesn't queue behind it
                        q = (nc.gpsimd if (hp == 1 and
                                           si == len(STRIPS) - 2)
                             else nc.sync)
                        q.dma_start(
                            OUT[:, out_col(off, hp):out_col(off, hp) + w],
                            stage[:, hp * w:(hp + 1) * w])

            for si, (off, w) in enumerate(STRIPS):
                for mc in range(MC):
                    qp = pQ.tile([128, SQW], f32, tag="qp")
                    for kc in range(KC):
                        nc.tensor.matmul(
                            qp[:, 0:w],
                            wqt[:, kc * DL + mc * 128:kc * DL + mc * 128 + 128],
                            xq[:, xq_col(off, kc):xq_col(off, kc) + w],
                            start=(kc == 0), stop=(kc == KC - 1))
                    if si == 0 and mc == 0:
                        emit_A(pend_A)
                        for hp in range(MC):
                            # only the per-head diagonal blocks (the
                            # cross-head blocks of A_ps are garbage and
                            # A_sb stays zero there), plus the host-
                            # folded overflow-key contribution
                            nc.vector.tensor_add(
                                A_sb[0:64, hp * 128:hp * 128 + 64],
                                A_ps[hp][0:64, 0:64],
                                kvmax[0:64, SKV + hp * 128:
                                      SKV + hp * 128 + 64])
                            nc.vector.tensor_add(
                                A_sb[64:128, hp * 128 + 64:(hp + 1) * 128],
                                A_ps[hp][64:128, 64:128],
                                kvmax[64:128, SKV + hp * 128 + 64:
                                      SKV + (hp + 1) * 128])
                    drain(qT[mc][:, off:off + w], qp[:, 0:w],
                          bias=(bq2[:, mc:mc + 1] if with_bias else None))
                if si > 0:
                    emit_ctx(si - 1)
            emit_ctx(len(STRIPS) - 1, split=True)

    nc.compile()
    return nc


def _get_nc(skv, with_bias):
    key = bool(with_bias)
    if key not in _cache:
        _cache[key] = _build(key)
    return _cache[key]


def _make_in_maps(hidden_states, attention_mask, Wq, bq, Wk, bk, Wv, bv):
    global _last_perms
    hs16 = np.asarray(hidden_states, dtype=np.float32).astype(np.float16)
    am = np.asarray(attention_mask, dtype=np.float32)
    bq = np.asarray(bq, np.float32)
    bk = np.asarray(bk, np.float32)
    bv = np.asarray(bv, np.float32)
    with_bias = bool(bq.any() or bk.any() or bv.any())

    valid = [np.nonzero(am[b, 0, 0, :] >= 0)[0] for b in range(B)]
    nmax = max(len(v) for v in valid)
    if nmax == 0:
        return None, with_bias, None   # all keys masked -> zero output

    xqs, xkvs, kvms, overs, perms = [], [], [], [], []
    for b in range(B):
        v = valid[b]
        inv = np.nonzero(am[b, 0, 0, :] < 0)[0]
        perm = np.concatenate([v, inv])          # valid keys first
        perms.append(perm)
        overs.append(v[CAP:])                    # folded host-side
        nv = min(len(v), CAP)
        kvm = (np.arange(CAP) < nv).astype(np.float32)
        kvms.append(np.ascontiguousarray(kvm.reshape(SKV, 128).T))
        xp = np.ascontiguousarray(hs16[b].T[:, perm])          # [D, S]
        xqs.append(np.ascontiguousarray(
            xp.reshape(KC, 128, NSQ, SQW).transpose(1, 2, 0, 3)
            .reshape(128, NSQ * KC * SQW)))
        xkvs.append(np.ascontiguousarray(
            xp[:, 0:512].reshape(KC, 128, 4, 128).transpose(1, 2, 0, 3)
            .reshape(128, 4 * KC * 128)))
    _last_perms = perms

    Wq = np.asarray(Wq, np.float32)
    Wk = np.asarray(Wk, np.float32)
    Wv = np.asarray(Wv, np.float32)

    def chunked(w):  # [1024, F] -> [128, KC*F] with col kc*F + m
        F = w.shape[1]
        return np.ascontiguousarray(
            w.reshape(KC, 128, F).transpose(1, 0, 2).reshape(128, KC * F)
            .astype(np.float16))

    in_maps = []
    for c in range(N_CORES):
        b, g = divmod(c, 4)
        sl = slice(g * DL, (g + 1) * DL)
        axm = np.zeros((128, MC * 128), np.float32)
        for j in overs[b]:
            x = hs16[b][j].astype(np.float32)
            kf = Wk[sl] @ x + bk[sl]
            vf = Wv[sl] @ x + bv[sl]
            for hp in range(MC):
                for h in range(2):
                    f = slice(hp * 128 + h * 64, hp * 128 + (h + 1) * 64)
                    axm[h * 64:(h + 1) * 64, f] += np.outer(kf[f], vf[f])
        wkv_img = chunked(np.concatenate([Wk[sl, :].T, Wv[sl, :].T],
                                         axis=1))
        c0_img = xkvs[b][:, 0:1024]
        m = {
            "xq": xqs[b],
            "xkv3": np.ascontiguousarray(xkvs[b][:, 1024:4096]),
            "kvhead": np.ascontiguousarray(np.concatenate(
                [np.concatenate([wkv_img[:, j * 1024:(j + 1) * 1024],
                                 c0_img[:, j * 256:(j + 1) * 256]], axis=1)
                 for j in range(4)], axis=1)),
            "wqt": chunked(Wq[sl, :].T),
            "kvmax": np.ascontiguousarray(
                np.concatenate([kvms[b], axm], axis=1)),
        }
        if with_bias:
            m["bq2"] = np.ascontiguousarray(bq[sl].reshape(MC, 128).T)
            m["bkv"] = np.ascontiguousarray(
                np.concatenate([bk[sl], bv[sl]]).reshape(1, 2 * DL)
                .astype(np.float16))
            m["ones"] = np.ones((1, 128), np.float16)
        in_maps.append(m)
    return SKV, with_bias, in_maps


def _gather(results):
    out = np.empty((B, S, D), np.float32)
    for c in range(N_CORES):
        b, g = divmod(c, 4)
        # out dram [128, NSQ*2*512]: [p, (s, hp, q)] -> [s*512+q, hp*128+p]
        arr = results[c]["out"].reshape(128, NSQ, MC, SQW)
        tmp = arr.transpose(1, 3, 2, 0).reshape(S, DL).astype(np.float32)
        out[b, _last_perms[b], g * DL:(g + 1) * DL] = tmp
    return out


def run_sharded(skv, with_bias, in_maps, **kw):
    nc = _get_nc(skv, with_bias)
    return run_bass_kernel_spmd(nc, in_maps, core_ids=list(range(N_CORES)),
                                **kw)


def kernel(hidden_states, attention_mask, Wq, bq, Wk, bk, Wv, bv):
    skv, with_bias, in_maps = _make_in_maps(
        hidden_states, attention_mask, Wq, bq, Wk, bk, Wv, bv)
    if skv is None:
        return np.zeros((B, S, D), np.float32)
    res = run_sharded(skv, with_bias, in_maps)
    return _gather(res.results)
